# revision 3
# baseline (speedup 1.0000x reference)
"""ConcatCritic all-pairs MLP scores on 8 Trainium2 NeuronCores.

scores[i, j] = MLP(concat(x[j], y[i])) computed as a [B, B] grid, sharded
by y-rows across 8 cores (each core computes a [B/8, B] slab).

Key restructure: layer 1 of the MLP acts on concat(x[j], y[i]), so
    z1[i, j, :] = x[j] @ W1x + (y[i] @ W1y + b1)
which is precomputed once as AT = (x @ W1x).T  [H, B] and
CT = (y_slab @ W1y + b1).T  [H, R].  Per row i, h1.T = relu(AT + CT[:, i])
is a single per-partition scalar add+max on the vector engine. This removes
the [B*B, 256] @ [256, 512] matmul entirely.

Layers 2/3 run on the tensor engine in float32r (FP22 multiplies, FP32
accumulate) at 1 cycle/row -- 4x the speed of true-fp32 matmuls with
~2^-12 relative precision.
"""

import threading

import numpy as np

B = 512
DX = 128
DY = 128
H = 512
P = 128
NCORES = 8
R = B // NCORES  # 64 rows of the pair grid per core
HB = H // P  # 4 partition-blocks of the hidden dim
JB = B // P  # 4 partition-blocks of the j axis
GS = 8  # output rows batched per store DMA

_cache_lock = threading.Lock()
_cached_nc = None


def _build_bass():
    """Emit the Bass/Tile program for one core's [R, B] slab."""
    import concourse.bass as bass  # noqa: F401
    import concourse.tile as tile
    from concourse import bacc, mybir
    from concourse.masks import make_identity

    f32 = mybir.dt.float32
    f32r = mybir.dt.float32r
    Relu = mybir.ActivationFunctionType.Relu
    add = mybir.AluOpType.add
    amax = mybir.AluOpType.max

    nc = bacc.Bacc(
        "TRN2",
        target_bir_lowering=False,
        debug=False,
        enable_asserts=False,
    )

    x_d = nc.dram_tensor("x", (B, DX), f32, kind="ExternalInput").ap()
    ys_d = nc.dram_tensor("ys", (R, DY), f32, kind="ExternalInput").ap()
    w1_d = nc.dram_tensor("w1", (DX + DY, H), f32r, kind="ExternalInput").ap()
    b1_d = nc.dram_tensor("b1", (H,), f32, kind="ExternalInput").ap()
    w2_d = nc.dram_tensor("w2", (H, H), f32r, kind="ExternalInput").ap()
    b2_d = nc.dram_tensor("b2", (H,), f32, kind="ExternalInput").ap()
    w3_d = nc.dram_tensor("w3", (H, 1), f32r, kind="ExternalInput").ap()
    b3_d = nc.dram_tensor("b3", (1,), f32, kind="ExternalInput").ap()
    out_d = nc.dram_tensor("s_slab", (R, B), f32, kind="ExternalOutput").ap()

    def r32(ap):
        return ap.bitcast(f32r)

    with tile.TileContext(nc) as tc:
        with (
            tc.tile_pool(name="const", bufs=1) as cpool,
            tc.tile_pool(name="h1p", bufs=3) as h1pool,
            tc.tile_pool(name="h2p", bufs=3) as h2pool,
            tc.tile_pool(name="sgp", bufs=2) as spool,
            tc.tile_pool(name="ps_l2", bufs=4, space="PSUM") as ps_l2,
            tc.tile_pool(name="ps_aux", bufs=2, space="PSUM") as ps_aux,
        ):
            # ---------------- constants / weights ----------------
            ident = cpool.tile([P, P], f32)
            make_identity(nc, ident)

            w1x = cpool.tile([P, H], f32r)  # [dx, h]
            nc.sync.dma_start(w1x[:], w1_d[:DX, :])
            w1y = cpool.tile([P, H], f32r)  # [dy, h]
            nc.sync.dma_start(w1y[:], w1_d[DX:, :])
            w2 = cpool.tile([P, HB, H], f32r)  # [p, kb, m]: W2[kb*P+p, m]
            nc.sync.dma_start(w2[:], w2_d.rearrange("(kb p) m -> p kb m", p=P))
            w3 = cpool.tile([P, HB], f32r)  # W3[kb*P+p, 0]
            nc.sync.dma_start(w3[:], w3_d.rearrange("(kb p) m -> p (kb m)", p=P))
            b1 = cpool.tile([P, HB], f32)
            nc.sync.dma_start(b1[:], b1_d.rearrange("(o p) -> p o", p=P))
            b2 = cpool.tile([P, HB], f32)
            nc.sync.dma_start(b2[:], b2_d.rearrange("(o p) -> p o", p=P))
            b3 = cpool.tile([1, 1], f32)
            nc.sync.dma_start(b3[:], b3_d[None, :])

            # x natural layout, then PE-transpose to xT [dx, j]
            x_sb = cpool.tile([P, JB, DX], f32)  # x[jb*P+p, d]
            nc.sync.dma_start(x_sb[:], x_d.rearrange("(jb p) d -> p jb d", p=P))
            xT = cpool.tile([P, B], f32r)  # [dx, j]
            for jb in range(JB):
                ps_t = ps_aux.tile([P, P], f32, tag="tr")
                nc.tensor.transpose(ps_t[:], x_sb[:, jb, :], ident[:])
                nc.vector.tensor_copy(xT[:, jb * P : (jb + 1) * P], ps_t[:])

            ys_sb = cpool.tile([R, DY], f32)
            nc.sync.dma_start(ys_sb[:], ys_d[:, :])
            yT = cpool.tile([P, R], f32r)  # [dy, i]
            ps_t = ps_aux.tile([P, P], f32, tag="tr")
            nc.tensor.transpose(ps_t[:, :R], ys_sb[:], ident[:R, :R])
            nc.vector.tensor_copy(yT[:], ps_t[:, :R])

            # AT[h, j] = (x @ W1x).T ; CTb[h, i] = (ys @ W1y).T + b1[h]
            at = cpool.tile([P, HB, B], f32)
            ctb = cpool.tile([P, HB, R], f32)
            for hb in range(HB):
                hsl = slice(hb * P, (hb + 1) * P)
                ps_a = ps_l2.tile([P, B], f32, tag="l2")
                nc.tensor.matmul(ps_a[:], w1x[:, hsl], xT[:])
                nc.vector.tensor_copy(at[:, hb, :], ps_a[:])
                ps_c = ps_aux.tile([P, P], f32, tag="tr")
                nc.tensor.matmul(ps_c[:, :R], w1y[:, hsl], yT[:])
                nc.vector.tensor_scalar_add(
                    ctb[:, hb, :], ps_c[:, :R], scalar1=b1[:, hb : hb + 1]
                )

            # ---------------- main loop over the R y-rows ----------------
            # Layer-3 matmuls for row r are emitted during row r+1's layer-2
            # matmuls so the tensor engine never waits on the scalar engine.
            h2_live = {}
            sg_live = {}
            for r in range(R + 1):
                if r < R:
                    # h1T = relu(AT + CTb[:, r])  (vector engine, one op/block)
                    h1 = h1pool.tile([P, HB, B], f32r, tag="h1")
                    for hb in range(HB):
                        nc.vector.tensor_scalar(
                            out=h1[:, hb, :],
                            in0=at[:, hb, :],
                            scalar1=ctb[:, hb, r : r + 1],
                            scalar2=0.0,
                            op0=add,
                            op1=amax,
                        )
                    # z2T = W2.T @ h1T ; h2T = relu(z2T + b2)
                    h2 = h2pool.tile([P, HB, B], f32r, tag="h2")
                    for mb in range(HB):
                        msl = slice(mb * P, (mb + 1) * P)
                        pl2 = ps_l2.tile([P, B], f32, tag="l2")
                        for kb in range(HB):
                            nc.tensor.matmul(
                                pl2[:],
                                w2[:, kb, msl],
                                h1[:, kb, :],
                                start=(kb == 0),
                                stop=(kb == HB - 1),
                            )
                        nc.scalar.activation(
                            h2[:, mb, :], pl2[:], Relu, bias=b2[:, mb : mb + 1]
                        )
                    h2_live[r] = h2

                rr = r - 1
                if rr >= 0:
                    # sT[rr, :] = W3.T @ h2T + b3  (M=1 matmuls)
                    h2p = h2_live.pop(rr)
                    ps_s = ps_aux.tile([1, B], f32, tag="s")
                    for kb in range(HB):
                        nc.tensor.matmul(
                            ps_s[:],
                            w3[:, kb : kb + 1],
                            h2p[:, kb, :],
                            start=(kb == 0),
                            stop=(kb == HB - 1),
                        )
                    g, gi = divmod(rr, GS)
                    if gi == 0:
                        sg_live[g] = spool.tile(
                            [1, GS, B], f32, tag="sg", name=f"sg_{g}"
                        )
                    nc.vector.tensor_scalar_add(
                        sg_live[g][:, gi, :], ps_s[:], scalar1=b3[:]
                    )
                    if gi == GS - 1:
                        sg = sg_live.pop(g)
                        nc.sync.dma_start(out_d[g * GS : (g + 1) * GS, :], sg[:])

    nc.compile()
    return nc


def _get_nc():
    global _cached_nc
    with _cache_lock:
        if _cached_nc is None:
            _cached_nc = _build_bass()
        return _cached_nc


def run(inputs, trace=False, **run_kwargs):
    """Shard, run on 8 cores, gather. Returns (out [B,B] f32, BassKernelResults)."""
    from concourse import bass_utils

    nc = _get_nc()
    x = np.ascontiguousarray(inputs["x"], dtype=np.float32)
    y = np.ascontiguousarray(inputs["y"], dtype=np.float32)
    common = {
        "x": x,
        "w1": np.ascontiguousarray(inputs["W1"], dtype=np.float32),
        "b1": np.ascontiguousarray(inputs["b1"], dtype=np.float32),
        "w2": np.ascontiguousarray(inputs["W2"], dtype=np.float32),
        "b2": np.ascontiguousarray(inputs["b2"], dtype=np.float32),
        "w3": np.ascontiguousarray(inputs["W3"], dtype=np.float32),
        "b3": np.ascontiguousarray(inputs["b3"], dtype=np.float32),
    }
    in_maps = [
        {**common, "ys": np.ascontiguousarray(y[d * R : (d + 1) * R])}
        for d in range(NCORES)
    ]
    res = bass_utils.run_bass_kernel_spmd(
        nc, in_maps, core_ids=list(range(NCORES)), trace=trace, **run_kwargs
    )
    s2 = np.concatenate([res.results[d]["s_slab"] for d in range(NCORES)], axis=0)
    return np.ascontiguousarray(s2.T), res


def kernel(**inputs) -> np.ndarray:
    out, _ = run(inputs, trace=False)
    return out


# revision 6
# speedup vs baseline: 2248.3858x; 2248.3858x over previous
"""ConcatCritic all-pairs MLP scores on 8 Trainium2 NeuronCores.

scores[i, j] = MLP(concat(x[j], y[i])) computed as a [B, B] grid, sharded
by y-rows across 8 cores (each core computes a [B/8, B] slab).

Key restructure: layer 1 of the MLP acts on concat(x[j], y[i]), so
    z1[i, j, :] = x[j] @ W1x + (y[i] @ W1y + b1)
which is precomputed once as AT = (x @ W1x).T  [H, B] and
CT = (y_slab @ W1y + b1).T  [H, R].  Per row i, h1.T = relu(AT + CT[:, i])
is a single per-partition scalar add+max on the vector engine. This removes
the [B*B, 256] @ [256, 512] matmul entirely.

Layers 2/3 run on the tensor engine in float32r (FP22 multiplies, FP32
accumulate) at 1 cycle/row -- 4x the speed of true-fp32 matmuls with
~2^-12 relative precision.
"""

import threading

import numpy as np

B = 512
DX = 128
DY = 128
H = 512
P = 128
NCORES = 8
R = B // NCORES  # 64 rows of the pair grid per core
HB = H // P  # 4 partition-blocks of the hidden dim
JB = B // P  # 4 partition-blocks of the j axis
GS = 8  # output rows batched per store DMA

_cache_lock = threading.Lock()
_cached_nc = {}


def _build_bass(nloop=1):
    """Emit the Bass/Tile program for one core's [R, B] slab."""
    import concourse.bass as bass  # noqa: F401
    import concourse.tile as tile
    from concourse import bacc, mybir
    from concourse.masks import make_identity

    f32 = mybir.dt.float32
    f32r = mybir.dt.float32r
    Relu = mybir.ActivationFunctionType.Relu
    add = mybir.AluOpType.add
    amax = mybir.AluOpType.max

    nc = bacc.Bacc(
        "TRN2",
        target_bir_lowering=False,
        debug=False,
        enable_asserts=False,
    )

    x_d = nc.dram_tensor("x", (B, DX), f32, kind="ExternalInput").ap()
    ys_d = nc.dram_tensor("ys", (R, DY), f32, kind="ExternalInput").ap()
    w1_d = nc.dram_tensor("w1", (DX + DY, H), f32r, kind="ExternalInput").ap()
    b1_d = nc.dram_tensor("b1", (H,), f32, kind="ExternalInput").ap()
    w2_d = nc.dram_tensor("w2", (H, H), f32r, kind="ExternalInput").ap()
    b2_d = nc.dram_tensor("b2", (H,), f32, kind="ExternalInput").ap()
    w3_d = nc.dram_tensor("w3", (H, 1), f32r, kind="ExternalInput").ap()
    b3_d = nc.dram_tensor("b3", (1,), f32, kind="ExternalInput").ap()
    out_d = nc.dram_tensor("s_slab", (R, B), f32, kind="ExternalOutput").ap()

    def r32(ap):
        return ap.bitcast(f32r)

    with tile.TileContext(nc) as tc:
        with (
            tc.tile_pool(name="const", bufs=1) as cpool,
            tc.tile_pool(name="h1p", bufs=3) as h1pool,
            tc.tile_pool(name="h2p", bufs=3) as h2pool,
            tc.tile_pool(name="sgp", bufs=2) as spool,
            tc.tile_pool(name="ps_l2", bufs=4, space="PSUM") as ps_l2,
            tc.tile_pool(name="ps_aux", bufs=2, space="PSUM") as ps_aux,
        ):
            # ---------------- constants / weights ----------------
            ident = cpool.tile([P, P], f32)
            make_identity(nc, ident)

            w1x = cpool.tile([P, H], f32r)  # [dx, h]
            nc.sync.dma_start(w1x[:], w1_d[:DX, :])
            w1y = cpool.tile([P, H], f32r)  # [dy, h]
            nc.sync.dma_start(w1y[:], w1_d[DX:, :])
            w2 = cpool.tile([P, HB, H], f32r)  # [p, kb, m]: W2[kb*P+p, m]
            nc.sync.dma_start(w2[:], w2_d.rearrange("(kb p) m -> p kb m", p=P))
            w3 = cpool.tile([P, HB], f32r)  # W3[kb*P+p, 0]
            nc.sync.dma_start(w3[:], w3_d.rearrange("(kb p) m -> p (kb m)", p=P))
            b1 = cpool.tile([P, HB], f32)
            nc.sync.dma_start(b1[:], b1_d.rearrange("(o p) -> p o", p=P))
            b2 = cpool.tile([P, HB], f32)
            nc.sync.dma_start(b2[:], b2_d.rearrange("(o p) -> p o", p=P))
            b3 = cpool.tile([1, 1], f32)
            nc.sync.dma_start(b3[:], b3_d[None, :])

            # x natural layout, then PE-transpose to xT [dx, j]
            x_sb = cpool.tile([P, JB, DX], f32)  # x[jb*P+p, d]
            nc.sync.dma_start(x_sb[:], x_d.rearrange("(jb p) d -> p jb d", p=P))
            xT = cpool.tile([P, B], f32r)  # [dx, j]
            for jb in range(JB):
                ps_t = ps_aux.tile([P, P], f32, tag="tr")
                nc.tensor.transpose(ps_t[:], x_sb[:, jb, :], ident[:])
                nc.vector.tensor_copy(xT[:, jb * P : (jb + 1) * P], ps_t[:])

            ys_sb = cpool.tile([R, DY], f32)
            nc.sync.dma_start(ys_sb[:], ys_d[:, :])
            yT = cpool.tile([P, R], f32r)  # [dy, i]
            ps_t = ps_aux.tile([P, P], f32, tag="tr")
            nc.tensor.transpose(ps_t[:, :R], ys_sb[:], ident[:R, :R])
            nc.vector.tensor_copy(yT[:], ps_t[:, :R])

            # AT[h, j] = (x @ W1x).T ; CTb[h, i] = (ys @ W1y).T + b1[h]
            at = cpool.tile([P, HB, B], f32)
            ctb = cpool.tile([P, HB, R], f32)
            for hb in range(HB):
                hsl = slice(hb * P, (hb + 1) * P)
                ps_a = ps_l2.tile([P, B], f32, tag="l2")
                nc.tensor.matmul(ps_a[:], w1x[:, hsl], xT[:])
                nc.vector.tensor_copy(at[:, hb, :], ps_a[:])
                ps_c = ps_aux.tile([P, P], f32, tag="tr")
                nc.tensor.matmul(ps_c[:, :R], w1y[:, hsl], yT[:])
                nc.vector.tensor_scalar_add(
                    ctb[:, hb, :], ps_c[:, :R], scalar1=b1[:, hb : hb + 1]
                )

            # ---------------- main loop over the R y-rows ----------------
            # Layer-3 matmuls for row r are emitted during row r+1's layer-2
            # matmuls so the tensor engine never waits on the scalar engine.
            h2_live = {}
            sg_live = {}
            for it in range(nloop):
              for r in range(R + 1):
                if r < R:
                    # h1T = relu(AT + CTb[:, r])  (vector engine, one op/block)
                    h1 = h1pool.tile([P, HB, B], f32r, tag="h1")
                    for hb in range(HB):
                        nc.vector.tensor_scalar(
                            out=h1[:, hb, :],
                            in0=at[:, hb, :],
                            scalar1=ctb[:, hb, r : r + 1],
                            scalar2=0.0,
                            op0=add,
                            op1=amax,
                        )
                    # z2T = W2.T @ h1T ; h2T = relu(z2T + b2)
                    h2 = h2pool.tile([P, HB, B], f32r, tag="h2")
                    for mb in range(HB):
                        msl = slice(mb * P, (mb + 1) * P)
                        pl2 = ps_l2.tile([P, B], f32, tag="l2")
                        for kb in range(HB):
                            nc.tensor.matmul(
                                pl2[:],
                                w2[:, kb, msl],
                                h1[:, kb, :],
                                start=(kb == 0),
                                stop=(kb == HB - 1),
                            )
                        nc.scalar.activation(
                            h2[:, mb, :], pl2[:], Relu, bias=b2[:, mb : mb + 1]
                        )
                    h2_live[r] = h2

                rr = r - 1
                if rr >= 0:
                    # sT[rr, :] = W3.T @ h2T + b3  (M=1 matmuls)
                    h2p = h2_live.pop(rr)
                    ps_s = ps_aux.tile([1, B], f32, tag="s")
                    for kb in range(HB):
                        nc.tensor.matmul(
                            ps_s[:],
                            w3[:, kb : kb + 1],
                            h2p[:, kb, :],
                            start=(kb == 0),
                            stop=(kb == HB - 1),
                        )
                    g, gi = divmod(rr, GS)
                    if gi == 0:
                        sg_live[g] = spool.tile(
                            [1, GS, B], f32, tag="sg", name=f"sg_{it}_{g}"
                        )
                    nc.vector.tensor_scalar_add(
                        sg_live[g][:, gi, :], ps_s[:], scalar1=b3[:]
                    )
                    if gi == GS - 1:
                        sg = sg_live.pop(g)
                        nc.sync.dma_start(out_d[g * GS : (g + 1) * GS, :], sg[:])

    nc.compile()
    return nc


def _get_nc(nloop=1):
    with _cache_lock:
        if nloop not in _cached_nc:
            _cached_nc[nloop] = _build_bass(nloop)
        return _cached_nc[nloop]


def run(inputs, trace=False, **run_kwargs):
    """Shard, run on 8 cores, gather. Returns (out [B,B] f32, BassKernelResults)."""
    from concourse import bass_utils

    nc = _get_nc()
    x = np.ascontiguousarray(inputs["x"], dtype=np.float32)
    y = np.ascontiguousarray(inputs["y"], dtype=np.float32)
    common = {
        "x": x,
        "w1": np.ascontiguousarray(inputs["W1"], dtype=np.float32),
        "b1": np.ascontiguousarray(inputs["b1"], dtype=np.float32),
        "w2": np.ascontiguousarray(inputs["W2"], dtype=np.float32),
        "b2": np.ascontiguousarray(inputs["b2"], dtype=np.float32),
        "w3": np.ascontiguousarray(inputs["W3"], dtype=np.float32),
        "b3": np.ascontiguousarray(inputs["b3"], dtype=np.float32),
    }
    in_maps = [
        {**common, "ys": np.ascontiguousarray(y[d * R : (d + 1) * R])}
        for d in range(NCORES)
    ]
    res = bass_utils.run_bass_kernel_spmd(
        nc, in_maps, core_ids=list(range(NCORES)), trace=trace, **run_kwargs
    )
    s2 = np.concatenate([res.results[d]["s_slab"] for d in range(NCORES)], axis=0)
    return np.ascontiguousarray(s2.T), res


def kernel(**inputs) -> np.ndarray:
    # One retry: the axon-tunneled cores occasionally throw a transient
    # NRT_EXEC_UNIT_UNRECOVERABLE on the first touch after an idle period.
    try:
        out, _ = run(inputs, trace=False)
    except Exception:  # noqa: BLE001
        import time as _time

        _time.sleep(2.0)
        out, _ = run(inputs, trace=False)
    return out
